# revision 29
# baseline (speedup 1.0000x reference)
"""Fused attention kernel v5 for nn_Attention_1090921693811, one sample per core.

v5 = v4 with a PE tile-config-uniform design (HW measurements showed ~170-200ns
penalty per PE tile-config switch):
- conv1x1 contraction split 96+96 (both round to 128-row config)
- qk channels permuted into head-pair blocks [q_h0 q_h1 k_h0 k_h1 | q_h2 ...]
  so route-B drains/transposes land pair-aligned in staging and the Gram runs
  as 2 pair-matmuls of [96,192]/[96,96] at full (128,128) config
- v re-tiled 96+96: T3=v[0:96] dense-DVE -> v_dw_a (no drain);
  T4=v[96:192] diag-stationary PE dw (96-row/96-col -> 128 config)
- tail: 96-row contraction passes against v_dw_a/v_dw_b
"""
import sys
sys.path.insert(0, '/opt/trn_rl_repo')
import numpy as np
from contextlib import ExitStack
from concourse import bass, bacc, mybir, tile

F32 = mybir.dt.float32
F32R = mybir.dt.float32r
FP16 = mybir.dt.float16
Alu = mybir.AluOpType
Act = mybir.ActivationFunctionType
AxX = mybir.AxisListType.X

C = 192; C3 = 576; HEADS = 4; CH = 48; H = 128; W = 128; N = H * W
R = 16                   # stripe output rows
NS = H // R              # stripes
SROWS = R + 2            # buffer rows incl halo
STRIDE = 130             # padded row stride: [128 data][2 pad]
ABUF = 2 + (SROWS + 1) * STRIDE
TOPKS = (24, 32, 36, 38)
NEG = -1e30

# output-channel tiles in PERMUTED order:
# qk part (0:384): [q_h0 q_h1 k_h0 k_h1 | q_h2 q_h3 k_h2 k_h3]
# v part (384:576): unpermuted
# T0=perm[0:128] T1=perm[128:256] T2=perm[256:384] T3=v[0:96] T4=v[96:192]
OT = [(0, 128), (128, 128), (256, 128), (384, 96), (480, 96)]

PERM = (list(range(0, 96)) + list(range(192, 288)) +
        list(range(96, 192)) + list(range(288, 384)) + list(range(384, 576)))


def host_prep(x, w_qkv, w_dw, w_proj, temperature, attn1, attn2, attn3, attn4):
    x = np.asarray(x, np.float32).reshape(C, N)
    wq = np.asarray(w_qkv, np.float32).reshape(3 * C, C)[PERM]
    wdw = np.asarray(w_dw, np.float32).reshape(3 * C, 9)[PERM]
    wp = np.asarray(w_proj, np.float32).reshape(C, C)
    temp = np.asarray(temperature, np.float32).reshape(HEADS)
    wgts = np.stack([np.float32(np.asarray(a).reshape(())) for a in
                     (attn1, attn2, attn3, attn4)])
    d = {"x": x, "wqkvT": np.ascontiguousarray(wq.T)}
    # route-B diag moving matrices for T0, T1 [128, 9*128]
    for i in (0, 1):
        o0, ow = OT[i]
        dg = np.zeros((128, 9 * 128), np.float16)
        for t in range(9):
            dg[np.arange(ow), t * 128 + np.arange(ow)] = wdw[o0:o0 + ow, t].astype(np.float16)
        d[f"diagB{i}"] = dg
    # dense tap weights for T2 (DVE), T3 (DVE)
    d["wt2"] = np.ascontiguousarray(wdw[256:384, :])
    d["wt3"] = np.ascontiguousarray(wdw[384:480, :])
    # T4 diag-stationary [96, 9*96]
    dg4 = np.zeros((96, 9 * 96), np.float16)
    for t in range(9):
        dg4[np.arange(96), t * 96 + np.arange(96)] = wdw[480:576, t].astype(np.float16)
    d["diagT4"] = dg4
    d["ident"] = np.eye(128, dtype=np.float16)
    # diag-extraction patterns: qq diag within [96, 2*192] pair blocks,
    # kk diag within [96, 2*96]
    P = np.zeros((96, 2 * 192), np.float32)
    P2 = np.zeros((96, 2 * 96), np.float32)
    for p in range(2):
        P[np.arange(96), p * 192 + np.arange(96)] = 1.0
        P2[np.arange(96), p * 96 + np.arange(96)] = 1.0
    d["dpat"] = P
    d["dpat2"] = P2
    wpt = np.zeros((CH, HEADS * C), np.float16)
    for h in range(HEADS):
        wpt[:, h * C:(h + 1) * C] = wp.T[h * CH:(h + 1) * CH, :].astype(np.float16)
    d["wprojT"] = wpt
    d["temp_rep"] = np.ascontiguousarray(np.broadcast_to(temp[None, :], (CH, HEADS))).astype(np.float32)
    d["wgt_rep"] = np.ascontiguousarray(np.broadcast_to(wgts[None, :], (CH, 4))).astype(np.float32)
    d["ones1"] = np.ones((1, CH), np.float32)
    return d


def build(debug=(), reps=1):
    nc = bacc.Bacc("TRN2", target_bir_lowering=False)
    E = {}
    specs = [("x", [C, N], F32R), ("wqkvT", [C, C3], F32R),
             ("diagB0", [128, 9 * 128], FP16), ("diagB1", [128, 9 * 128], FP16),
             ("wt2", [128, 9], F32), ("wt3", [96, 9], F32),
             ("diagT4", [96, 9 * 96], FP16),
             ("ident", [128, 128], FP16), ("dpat", [96, 2 * 192], F32),
             ("dpat2", [96, 2 * 96], F32),
             ("wprojT", [CH, HEADS * C], FP16),
             ("temp_rep", [CH, HEADS], F32), ("wgt_rep", [CH, 4], F32),
             ("ones1", [1, CH], F32)]
    for name, shape, dt in specs:
        E[name] = nc.declare_dram_parameter(name, shape, dt, isOutput=False)
    out_ext = nc.declare_dram_parameter("out", [C, N], F32, isOutput=True)
    dbg_ext = {name: nc.declare_dram_parameter("dbg_" + name, list(shape), F32, isOutput=True)
               for name, shape in debug}
    dbg = dict(debug)

    with tile.TileContext(nc) as tc, ExitStack() as ctx:
        persist = ctx.enter_context(tc.tile_pool(name="persist", bufs=1))
        wqkvT = [persist.tile([96, C3], F32R, tag="wq0", name="wq0"),
                 persist.tile([96, C3], F32R, tag="wq1", name="wq1")]
        nc.sync.dma_start(wqkvT[0][:], E["wqkvT"][0:96, :])
        nc.sync.dma_start(wqkvT[1][:], E["wqkvT"][96:192, :])
        diagB = []
        for i in (0, 1):
            t_ = persist.tile([128, 9 * 128], FP16, tag=f"dB{i}", name=f"dB{i}")
            (nc.scalar, nc.gpsimd)[i].dma_start(t_[:], E[f"diagB{i}"][:])
            diagB.append(t_)
        wt2 = persist.tile([128, 9], F32, tag="wt2", name="wt2")
        wt3 = persist.tile([96, 9], F32, tag="wt3", name="wt3")
        diagT4 = persist.tile([96, 9 * 96], FP16, tag="dg4", name="dg4")
        ident = persist.tile([128, 128], FP16, tag="id", name="id")
        dpat = persist.tile([96, 2 * 192], F32, tag="dpat", name="dpat")
        dpat2 = persist.tile([96, 2 * 96], F32, tag="dpat2", name="dpat2")
        wprojT = persist.tile([CH, HEADS * C], FP16, tag="wpt", name="wpt")
        temp_rep = persist.tile([CH, HEADS], F32, tag="tmp_r", name="tmp_r")
        wgt_rep = persist.tile([CH, 4], F32, tag="wgt_r", name="wgt_r")
        ones1 = persist.tile([1, CH], F32, tag="on1", name="on1")
        for j, (t_, name) in enumerate(((wt2, "wt2"), (wt3, "wt3"), (diagT4, "diagT4"),
                                        (ident, "ident"), (dpat, "dpat"), (dpat2, "dpat2"),
                                        (wprojT, "wprojT"), (temp_rep, "temp_rep"),
                                        (wgt_rep, "wgt_rep"), (ones1, "ones1"))):
            (nc.scalar, nc.gpsimd)[j % 2].dma_start(t_[:], E[name][:])
        mid = ctx.enter_context(tc.tile_pool(name="mid", bufs=1))
        for _rep in range(reps):
            _run_once(nc, tc, mid, E, out_ext, dbg_ext, dbg if _rep == reps - 1 else {},
                      wqkvT, diagB, wt2, wt3, diagT4, ident, dpat, dpat2, wprojT,
                      temp_rep, wgt_rep, ones1, persist)
    nc.finalize()
    return nc


def _run_once(nc, tc, mid, E, out_ext, dbg_ext, dbg, wqkvT, diagB, wt2, wt3,
              diagT4, ident, dpat, dpat2, wprojT, temp_rep, wgt_rep, ones1, persist):
    v_dw = [persist.tile([96, N], FP16, tag="vdwa", name="vdwa"),
            persist.tile([96, N], FP16, tag="vdwb", name="vdwb")]

    def load_x_stripe(s, pool):
        r0 = max(s * R - 1, 0)
        r1 = min(s * R + R + 1, H)
        br0 = r0 - (s * R - 1)
        nr = r1 - r0
        xa = pool.tile([96, SROWS * W], F32R, tag="xa", name="xa")
        xb = pool.tile([96, SROWS * W], F32R, tag="xb", name="xb")
        nc.sync.dma_start(xa[:, br0 * W:(br0 + nr) * W], E["x"][0:96, r0 * W:r1 * W])
        nc.sync.dma_start(xb[:, br0 * W:(br0 + nr) * W], E["x"][96:192, r0 * W:r1 * W])
        return xa, xb

    drain_par = [0]  # PSUM-drain rotation: 2x ACT : 1x DVE

    def a_drain(dstA, srcP):
        if drain_par[0] % 4 < 3:
            nc.scalar.copy(dstA, srcP)
        else:
            nc.vector.tensor_copy(dstA, srcP)
        drain_par[0] += 1

    # ======================= main stripe loop =======================
    with tc.tile_pool(name="p1x", bufs=2) as xp, \
         tc.tile_pool(name="p1gps", bufs=3, space="PSUM") as gps, \
         tc.tile_pool(name="p1ab", bufs=7) as abp, \
         tc.tile_pool(name="p1d2", bufs=2) as d2p, \
         tc.tile_pool(name="p1stg", bufs=3) as stgp, \
         tc.tile_pool(name="qps", bufs=2, space="PSUM") as qps, \
         tc.tile_pool(name="tps", bufs=1, space="PSUM") as tps, \
         tc.tile_pool(name="gramp", bufs=1, space="PSUM") as gram_pool:
        gram_ps = gram_pool.tile([96, 2 * 192], F32, tag="g1", name="gram_ps")
        gram2_ps = gram_pool.tile([96, 2 * 96], F32, tag="g2", name="gram2_ps")
        nc.vector.memset(gram_ps[:], 0.0)
        nc.vector.memset(gram2_ps[:], 0.0)
        for s in range(NS):
            xa, xb = load_x_stripe(s, xp)
            As = [None] * 5
            # ---- conv1x1 for all 5 o-tiles -> padded fp16 A-bufs ----
            for i in range(5):
                o0, ow = OT[i]
                A = abp.tile([128, ABUF], FP16, tag="A", name=f"A{i}")
                nc.gpsimd.memset(A[:ow, 0:2], 0.0)
                nc.gpsimd.memset(A[:ow, 2:2 + SROWS * STRIDE].rearrange(
                    "p (r c) -> p r c", c=STRIDE)[:, :, 128:130], 0.0)
                if s == 0:
                    nc.gpsimd.memset(A[:ow, 2:2 + W], 0.0)
                if s == NS - 1:
                    nc.gpsimd.memset(A[:ow, 2 + (SROWS - 1) * STRIDE:2 + (SROWS - 1) * STRIDE + W], 0.0)
                ncols = SROWS * W
                c_lo = W if s == 0 else 0
                c_hi = (SROWS - 1) * W if s == NS - 1 else ncols
                for g0 in range(c_lo, c_hi, 512):
                    gw = min(512, c_hi - g0)
                    pg = gps.tile([128, 512], F32, tag="g", name="g")
                    for mi, (mt, xs) in enumerate(((wqkvT[0], xa), (wqkvT[1], xb))):
                        nc.tensor.matmul(pg[:ow, 0:gw], mt[:, o0:o0 + ow],
                                         xs[:, g0:g0 + gw], start=(mi == 0), stop=(mi == 1))
                    rr, nrow = g0 // W, gw // W
                    dstA = A[:ow, 2 + rr * STRIDE:2 + (rr + nrow) * STRIDE].rearrange(
                        "p (r c) -> p r c", c=STRIDE)[:, :, 0:128]
                    srcP = pg[:ow, 0:gw].rearrange("p (r c) -> p r c", c=W)
                    a_drain(dstA, srcP)
                As[i] = A

            # ---- T2 dense dw on DVE -> dense_T2; T3 dense dw on DVE -> v_dw_a ----
            dense_T2 = d2p.tile([128, R * W], FP16, tag="d2", name="d2")
            for (ti, A_, wsb, full_ap, ow, blks) in (
                    (2, As[2], wt2, dense_T2, 128, ((0, 4), (4, 4), (8, 4), (12, 4))),
                    (3, As[3], wt3, None, 96, ((0, 16),))):
                ntmp = 0
                for (r0, nr) in blks:
                    if ti == 2:
                        out_ap = full_ap[:, r0 * W:(r0 + nr) * W]
                    else:
                        out_ap = v_dw[0][:, (s * R + r0) * W:(s * R + r0 + nr) * W]

                    def src_ap(buf, base):
                        return buf[:ow, base + r0 * STRIDE:base + (r0 + nr) * STRIDE].rearrange(
                            "p (r c) -> p r c", c=STRIDE)[:, :, 0:128]
                    d3 = out_ap.rearrange("p (r c) -> p r c", c=W)
                    nc.vector.tensor_scalar(d3, src_ap(A_, 2 + STRIDE), wsb[:ow, 4:5],
                                            None, Alu.mult)
                    for dy in (-1, 0, 1):
                        for dx in (-1, 0, 1):
                            if dy == 0 and dx == 0:
                                continue
                            t = (dy + 1) * 3 + (dx + 1)
                            sap = src_ap(A_, 2 + (1 + dy) * STRIDE + dx)
                            tmp = d2p.tile([128, R * W], FP16, tag=f"tm{ntmp % 2}",
                                           name="tmp", bufs=2)
                            ntmp += 1
                            dap = tmp[:ow, 0:nr * W].rearrange("p (r c) -> p r c", c=W)
                            if t in (0, 6):
                                nc.scalar.activation(dap, sap, Act.Copy, bias=0.0,
                                                     scale=wsb[:ow, t:t + 1])
                            else:
                                nc.vector.tensor_scalar(dap, sap, wsb[:ow, t:t + 1],
                                                        None, Alu.mult)
                            nc.vector.tensor_tensor(out_ap, out_ap, tmp[:ow, 0:nr * W], Alu.add)

            # ---- T0,T1 route-B + T2 transposes + staging + gram (pair-packed) ----
            for g in range(R // 4):          # 4-row groups
                stg = stgp.tile([128, 4 * 384], FP16, tag="stg", name="stg")
                for i in (0, 1):
                    qp = qps.tile([128, 512], F32, tag="qp", name="qp")
                    for r in range(4):
                        br = g * 4 + r + 1
                        for t in range(9):
                            dy, dx = t // 3 - 1, t % 3 - 1
                            base = 2 + (br + dy) * STRIDE + dx
                            nc.tensor.matmul(qp[:, r * 128:(r + 1) * 128],
                                             As[i][:, base:base + 128],
                                             diagB[i][:, t * 128:(t + 1) * 128],
                                             start=(t == 0), stop=(t == 8),
                                             skip_group_check=True)
                    dst = stg[:].rearrange("p (r c) -> p r c", c=384)[:, :, i * 128:(i + 1) * 128]
                    nc.scalar.copy(dst, qp[:].rearrange("p (r c) -> p r c", c=128))
                pt = tps.tile([128, 512], FP16, tag="pt", name="pt")
                for r in range(4):
                    nc.tensor.transpose(pt[:, r * 128:r * 128 + 128],
                                        dense_T2[:, (g * 4 + r) * 128:(g * 4 + r + 1) * 128],
                                        ident[:])
                dstT = stg[:].rearrange("p (r c) -> p r c", c=384)[:, :, 256:384]
                nc.scalar.copy(dstT, pt[:].rearrange("p (r c) -> p r c", c=128))
                last_g = (s == NS - 1 and g == R // 4 - 1)
                for r in range(4):
                    stop = last_g and r == 3
                    for p in range(2):
                        qp_sl = stg[:, r * 384 + p * 192:r * 384 + p * 192 + 96]
                        qk_sl = stg[:, r * 384 + p * 192:r * 384 + (p + 1) * 192]
                        kp_sl = stg[:, r * 384 + p * 192 + 96:r * 384 + (p + 1) * 192]
                        nc.tensor.matmul(gram_ps[:, p * 192:(p + 1) * 192],
                                         qp_sl, qk_sl,
                                         start=False, stop=stop, skip_group_check=True)
                        nc.tensor.matmul(gram2_ps[:, p * 96:(p + 1) * 96],
                                         kp_sl, kp_sl,
                                         start=False, stop=stop, skip_group_check=True)
            # ---- T4: diag-stationary dw on PE -> v_dw_b (96ch, 128-config) ----
            A4 = As[4]
            for ch0 in range(0, R, 4):
                pv = qps.tile([128, 512], F32, tag="qp", name="pv")
                for t in range(9):
                    dy, dx = t // 3 - 1, t % 3 - 1
                    base = 2 + (1 + ch0 + dy) * STRIDE + dx
                    mov = A4[0:96, base:base + 4 * STRIDE].rearrange(
                        "p (r c) -> p r c", c=STRIDE)[:, :, 0:128]
                    nc.tensor.matmul(pv[:96, :].rearrange("p (r c) -> p r c", c=W),
                                     diagT4[:, t * 96:(t + 1) * 96], mov,
                                     start=(t == 0), stop=(t == 8), skip_group_check=True)
                c0_ = (s * R + ch0) * W
                a_drain(v_dw[1][:, c0_:c0_ + 4 * W], pv[:96, :])

        gram_sb3 = mid.tile([96, 2 * 192], F32, tag="g3", name="gram_sb3")
        gram_sb2 = mid.tile([96, 2 * 96], F32, tag="g2s", name="gram_sb2")
        nc.vector.tensor_copy(gram_sb3[:], gram_ps[:])
        nc.vector.tensor_copy(gram_sb2[:], gram2_ps[:])

    # ======================= MID: softmax etc =======================
    if True:
        if "gram3" in dbg:
            nc.sync.dma_start(dbg_ext["gram3"][:], gram_sb3[:])
        # norms^2 from qq/kk diagonals
        junk3 = mid.tile([96, 2 * 192], F32, tag="j3", name="junk3")
        junk2 = mid.tile([96, 2 * 96], F32, tag="j2", name="junk2")
        nc.vector.tensor_tensor(junk3[:], gram_sb3[:], dpat[:], Alu.mult)
        nc.vector.tensor_tensor(junk2[:], gram_sb2[:], dpat2[:], Alu.mult)
        rn_q = mid.tile([96, 2], F32, tag="rnq", name="rn_q")
        rn_k = mid.tile([96, 2], F32, tag="rnk", name="rn_k")
        for p in range(2):
            nc.vector.tensor_reduce(rn_q[:, p:p + 1], junk3[:, p * 192:(p + 1) * 192],
                                    AxX, Alu.add)
            nc.vector.tensor_reduce(rn_k[:, p:p + 1], junk2[:, p * 96:(p + 1) * 96],
                                    AxX, Alu.add)
        rn_q2 = mid.tile([96, 2], F32, tag="rnq2", name="rn_q2")
        rn_k2 = mid.tile([96, 2], F32, tag="rnk2", name="rn_k2")
        nc.scalar.sqrt(rn_q2[:], rn_q[:])
        nc.scalar.sqrt(rn_k2[:], rn_k[:])
        nc.vector.reciprocal(rn_q[:], rn_q2[:])
        nc.vector.reciprocal(rn_k[:], rn_k2[:])
        rqk = mid.tile([CH, 8], F32, tag="rqk", name="rqk")
        # head h: pair p=h//2, half j=h%2 (j=1 rows 48:96 -> partition-move DMA)
        for h in range(HEADS):
            p, j = h // 2, h % 2
            if j == 0:
                nc.vector.tensor_copy(rqk[:, h:h + 1], rn_q[0:48, p:p + 1])
                nc.vector.tensor_copy(rqk[:, 4 + h:5 + h], rn_k[0:48, p:p + 1])
            else:
                nc.sync.dma_start(rqk[:, h:h + 1], rn_q[48:96, p:p + 1])
                nc.sync.dma_start(rqk[:, 4 + h:5 + h], rn_k[48:96, p:p + 1])
        # attn blocks: head (p, j): gram_sb3[j*48:(j+1)*48, p*192+96+j*48 : +48]
        gram_sb = mid.tile([CH, HEADS * CH], F32, tag="gramsb", name="gram_sb")
        for h in range(HEADS):
            p, j = h // 2, h % 2
            src = gram_sb3[j * 48:(j + 1) * 48, p * 192 + 96 + j * 48:p * 192 + 96 + (j + 1) * 48]
            if j == 0:
                nc.vector.tensor_copy(gram_sb[:, h * CH:(h + 1) * CH], src)
            else:
                nc.sync.dma_start(gram_sb[:, h * CH:(h + 1) * CH], src)
        rk_row = mid.tile([1, HEADS * CH], F32, tag="rkrow", name="rkrow")
        for h in range(HEADS):
            nc.sync.dma_start(rk_row[0:1, h * CH:(h + 1) * CH], rqk[:, 4 + h:5 + h])
        with tc.tile_pool(name="midps", bufs=1, space="PSUM") as mps:
            rk_rep_ps = mps.tile([CH, HEADS * CH], F32, tag="m", name="rkrep")
            for h in range(HEADS):
                nc.tensor.matmul(rk_rep_ps[:, h * CH:(h + 1) * CH], ones1[:],
                                 rk_row[0:1, h * CH:(h + 1) * CH], start=True, stop=True)
            attn = mid.tile([CH, HEADS * CH], F32, tag="attn", name="attn")
            nc.vector.tensor_tensor(attn[:], gram_sb[:], rk_rep_ps[:], Alu.mult)
            if "gram" in dbg:
                nc.sync.dma_start(dbg_ext["gram"][:], gram_sb[:])
            s_col = mid.tile([CH, HEADS], F32, tag="scol", name="scol")
            nc.vector.tensor_tensor(s_col[:], rqk[:, 0:4], temp_rep[:], Alu.mult)
            srt = mid.tile([CH, 5 * 8], F32, tag="srt", name="srt")
            scratch = mid.tile([CH, HEADS * CH], F32, tag="scr", name="scr")
            e_t = mid.tile([CH, HEADS * CH], F32, tag="e", name="e")
            acc_m = mid.tile([CH, HEADS * CH], F32, tag="accm", name="accm")
            mx = mid.tile([CH, 8], F32, tag="mx", name="mx")
            sk = mid.tile([CH, 4], F32, tag="sk", name="sk")
            cf = mid.tile([CH, 4], F32, tag="cf", name="cf")
            junk = mid.tile([CH, CH], F32, tag="junk", name="junk")
            for h in range(HEADS):
                ah = attn[:, h * CH:(h + 1) * CH]
                sc = scratch[:, h * CH:(h + 1) * CH]
                nc.vector.tensor_copy(sc, ah)
                for it in range(5):
                    nc.vector.max(srt[:, it * 8:(it + 1) * 8], sc)
                    if it < 4:
                        nc.vector.match_replace(sc, srt[:, it * 8:(it + 1) * 8], sc, NEG)
                nc.vector.tensor_scalar(mx[:, h:h + 1], srt[:, 0:1], s_col[:, h:h + 1],
                                        -1.0, Alu.mult, Alu.mult)
                eh = e_t[:, h * CH:(h + 1) * CH]
                nc.scalar.activation(eh, ah, Act.Exp, bias=mx[:, h:h + 1], scale=s_col[:, h:h + 1])
                for ki, kk in enumerate(TOPKS):
                    th = srt[:, kk - 1:kk]
                    nc.vector.scalar_tensor_tensor(junk[:], ah, th, eh, Alu.is_ge, Alu.mult,
                                                   accum_out=sk[:, ki:ki + 1])
                nc.vector.reciprocal(sk[:], sk[:])
                nc.vector.tensor_tensor(cf[:], sk[:], wgt_rep[:], Alu.mult)
                am = acc_m[:, h * CH:(h + 1) * CH]
                for ki, kk in enumerate(TOPKS):
                    th = srt[:, kk - 1:kk]
                    if ki == 0:
                        nc.vector.tensor_scalar(am, ah, th, cf[:, ki:ki + 1], Alu.is_ge, Alu.mult)
                    else:
                        nc.vector.tensor_scalar(junk[:], ah, th, cf[:, ki:ki + 1], Alu.is_ge, Alu.mult)
                        nc.vector.tensor_tensor(am, am, junk[:], Alu.add)
                nc.vector.tensor_tensor(am, am, eh, Alu.mult)
            a_bf = mid.tile([CH, HEADS * CH], FP16, tag="abf", name="abf")
            nc.vector.tensor_copy(a_bf[:], acc_m[:])
            mh_sb = mid.tile([CH, HEADS * C], FP16, tag="mhsb", name="mhsb")
            for h in range(HEADS):
                mh_ps = mps.tile([CH, C], F32, tag="m", name="mh_ps")
                nc.tensor.matmul(mh_ps[:], a_bf[:, h * CH:(h + 1) * CH],
                                 wprojT[:, h * C:(h + 1) * C], start=True, stop=True)
                nc.vector.tensor_copy(mh_sb[:, h * C:(h + 1) * C], mh_ps[:])
            if "attn" in dbg:
                nc.sync.dma_start(dbg_ext["attn"][:], attn[:])
            if "accm" in dbg:
                nc.sync.dma_start(dbg_ext["accm"][:], acc_m[:])
            if "rqk" in dbg:
                nc.sync.dma_start(dbg_ext["rqk"][:], rqk[:])
        # mhatT: [96, C] x 2; head h -> mhatT[h//2] rows (h%2)*48:+48
        # row r of mhatT[p] = v-channel (p*96 + r), col = output channel
        mhatT = [mid.tile([96, C], FP16, tag="mhs0", name="mhs0"),
                 mid.tile([96, C], FP16, tag="mhs1", name="mhs1")]
        for h in range(HEADS):
            p, j = h // 2, h % 2
            nc.sync.dma_start(mhatT[p][j * 48:(j + 1) * 48, :], mh_sb[:, h * C:(h + 1) * C])

    # ======================= out = mhatT.T @ v_dw =======================
    with tc.tile_pool(name="p2o", bufs=4) as op, \
         tc.tile_pool(name="p2ops", bufs=3, space="PSUM") as ops_:
        ndr = 0
        dma_engs = (nc.sync, nc.gpsimd)
        for oo0, oow in ((0, 128), (128, 64)):
            for n0 in range(0, N, 1024):
                po = ops_.tile([128, 1024], F32, tag="o", name="o")
                for c0 in (0, 512):
                    nc.tensor.matmul(po[:oow, c0:c0 + 512], mhatT[0][:, oo0:oo0 + oow],
                                     v_dw[0][:, n0 + c0:n0 + c0 + 512], start=True, stop=False)
                    nc.tensor.matmul(po[:oow, c0:c0 + 512], mhatT[1][:, oo0:oo0 + oow],
                                     v_dw[1][:, n0 + c0:n0 + c0 + 512], start=False, stop=True)
                ot = op.tile([128, 1024], F32, tag="ot", name="ot")
                if ndr % 2 == 0:
                    nc.scalar.copy(ot[:oow, :], po[:oow, :])
                else:
                    nc.vector.tensor_copy(ot[:oow, :], po[:oow, :])
                dma_engs[ndr % 2].dma_start(out_ext[oo0:oo0 + oow, n0:n0 + 1024], ot[:oow, :])
                ndr += 1


from concourse.bass_utils import run_bass_kernel_spmd

B = 8
_CACHE = {}


def kernel(**inputs):
    """Full (unsharded) inputs -> full output [8, 192, 128, 128] float32.

    Shards the batch across 8 NeuronCores (one sample per core, pure data
    parallelism), runs the fused Bass kernel SPMD, gathers results.
    """
    x = np.asarray(inputs["x"], np.float32)
    if "nc" not in _CACHE:
        _CACHE["nc"] = build()
    nc = _CACHE["nc"]
    in_maps = [host_prep(x[b], inputs["w_qkv"], inputs["w_dw"], inputs["w_proj"],
                         inputs["temperature"], inputs["attn1"], inputs["attn2"],
                         inputs["attn3"], inputs["attn4"]) for b in range(B)]
    res = run_bass_kernel_spmd(nc, in_maps, list(range(B)))
    out = np.stack([res.results[b]["out"].reshape(C, H, W) for b in range(B)])
    return out.astype(np.float32)

